# revision 2
# baseline (speedup 1.0000x reference)
"""GCN encoder (3-layer GCNConv + LayerNorm + ReLU + residual) on 8 TRN2
NeuronCores via Bass/Tile.

Sharding: nodes are partitioned across cores (graph parallel). Each core owns
NPC nodes; per-layer the full (dinv-scaled) xw table is AllGathered to every
core's DRAM, then each core pulls its in-edge source rows with dma_gather,
scales by edge weight, and segment-reduces into its owned destinations.
"""

import numpy as np

import concourse.bacc as bacc
import concourse.bass as bass
import concourse.mybir as mybir
from concourse.tile import TileContext
from concourse.bass_utils import run_bass_kernel_spmd

F32 = mybir.dt.float32
I32 = mybir.dt.int32
AX = mybir.AxisListType
ALU = mybir.AluOpType
ACTF = mybir.ActivationFunctionType


# ----------------------------------------------------------------------------
# Host-side structure packing (pure index/layout manipulation + reordering)
# ----------------------------------------------------------------------------

def build_structure(edge_index, N, C, W, HALF=32768):
    """Partition nodes across C cores, degree-sort each core's dests into
    windows of 128, and build padded-CSR metadata.

    Returns a dict with per-core packing info plus the shared per-window K
    values (maxed over cores so the SPMD program is identical on all cores).
    """
    NPC = N // C              # owned (real) nodes per core
    NP = W * 128              # padded nodes per core
    src = edge_index[0].astype(np.int64)
    dst = edge_index[1].astype(np.int64)
    E = src.shape[0]

    # append self loops (weight handled separately by caller: w=1)
    loop = np.arange(N, dtype=np.int64)
    src2 = np.concatenate([src, loop])
    dst2 = np.concatenate([dst, loop])
    eid2 = np.arange(E + N, dtype=np.int64)   # index into w2 = [edge_weight, ones]

    owner = dst2 // NPC                        # dest core of each edge
    deg_all = np.bincount(dst2, minlength=N)   # per-dest slot count (incl self)

    # per-core permutation: sort owned dests by degree desc (stable)
    rank = np.empty(N, dtype=np.int64)         # local rank of node on its owner
    for c in range(C):
        lo, hi = c * NPC, (c + 1) * NPC
        order = np.argsort(-deg_all[lo:hi], kind="stable")
        rank[lo + order] = np.arange(NPC)
    node_pos = (np.arange(N) // NPC) * NP + rank      # table row of each node

    cores = []
    KA = np.zeros((C, W), dtype=np.int64)
    KB = np.zeros((C, W), dtype=np.int64)
    for c in range(C):
        sel = owner == c
        e_src = src2[sel]
        e_dst = dst2[sel]
        e_id = eid2[sel]
        dloc = rank[e_dst]                    # local dest rank [0, NPC)
        spos = node_pos[e_src]                # table row of source
        isB = (spos >= HALF).astype(np.int64)
        # sort by (dest rank, phase)
        o = np.lexsort((isB, dloc))
        dloc, spos, isB, e_id = dloc[o], spos[o], isB[o], e_id[o]
        cntA = np.bincount(dloc, weights=1 - isB, minlength=NP).astype(np.int64)
        cntB = np.bincount(dloc, weights=isB, minlength=NP).astype(np.int64)
        starts = np.zeros(NP, dtype=np.int64)
        starts[1:] = np.cumsum(cntA + cntB)[:-1]
        vw = np.arange(NP) // 128
        for w in range(W):
            m = vw == w
            KA[c, w] = cntA[m].max() if m.any() else 0
            KB[c, w] = cntB[m].max() if m.any() else 0
        cores.append(dict(dloc=dloc, spos=spos, isB=isB, eid=e_id,
                          cntA=cntA, cntB=cntB, starts=starts))

    KA = KA.max(axis=0)
    KB = KB.max(axis=0)
    return dict(NPC=NPC, NP=NP, HALF=HALF, C=C, W=W, KA=KA, KB=KB,
                cores=cores, rank=rank, node_pos=node_pos)


def _pad_block(vals, starts, lens, K, fill):
    """[128] ragged segments of `vals` -> padded [128, K] with `fill`."""
    col = np.arange(K)[None, :]
    mask = col < lens[:, None]
    sp = starts[:, None] + col
    sp = np.where(mask, sp, 0)
    out = np.where(mask, vals[sp], fill)
    return out


def pack_core(st, c, w2):
    """Build the int16 index image and f32 weight image for core c.

    Layout per window w: phase A block [128, KA[w]] then phase B block
    [128, KB[w]], concatenated along free dim over all windows.
    idx image: flat k-major wrap -> [128, 8*K] int16 per block.
    """
    W, KA, KB, HALF = st["W"], st["KA"], st["KB"], st["HALF"]
    d = st["cores"][c]
    dloc, spos, isB, eid = d["dloc"], d["spos"], d["isB"], d["eid"]
    cntA, cntB, starts = d["cntA"], d["cntB"], d["starts"]
    wvals = w2[eid]

    idx_cols = []
    w_cols = []
    for w in range(W):
        vs = slice(w * 128, (w + 1) * 128)
        saw = starts[vs]
        caw = cntA[vs]
        cbw = cntB[vs]
        for K, stt, ln, off in ((int(KA[w]), saw, caw, 0),
                                (int(KB[w]), saw + caw, cbw, HALF)):
            if K == 0:
                continue
            pi = _pad_block(spos, stt, ln, K, off).astype(np.int64) - off
            pw = _pad_block(wvals, stt, ln, K, 0.0)
            assert pi.min() >= 0
            idx_cols.append(pi.astype(np.int32))          # [128, K]
            w_cols.append(pw.astype(np.float32))          # [128, K]
    idx_img = np.concatenate(idx_cols, axis=1)
    w_img = np.concatenate(w_cols, axis=1)
    return idx_img, w_img


# ----------------------------------------------------------------------------
# Bass program
# ----------------------------------------------------------------------------

def build_program(st, L, D=128):
    W = st["W"]
    NP = st["NP"]
    HALF = st["HALF"]
    C = st["C"]
    KA, KB = st["KA"], st["KB"]
    KT = [int(KA[w] + KB[w]) for w in range(W)]
    KCOLS = int(sum(KT))
    IDXCOLS = KCOLS
    NT = NP * C                     # table rows

    nc = bacc.Bacc("TRN2", target_bir_lowering=False, debug=True)

    x_in = nc.dram_tensor("x_shard", [NP, D], F32, kind="ExternalInput")
    idx_in = nc.dram_tensor("idx_img", [128, IDXCOLS], I32, kind="ExternalInput")
    w_in = nc.dram_tensor("w_img", [128, KCOLS], F32, kind="ExternalInput")
    wst_in = nc.dram_tensor("wst", [L, D, D], F32, kind="ExternalInput")
    bias_in = nc.dram_tensor("bias_b", [L, D, D], F32, kind="ExternalInput")
    gam_in = nc.dram_tensor("gamma_b", [L, D, D], F32, kind="ExternalInput")
    bet_in = nc.dram_tensor("beta_b", [L, D, D], F32, kind="ExternalInput")
    id_in = nc.dram_tensor("ident", [D, D], F32, kind="ExternalInput")
    out_t = nc.dram_tensor("out_shard", [NP, D], F32, kind="ExternalOutput")

    with TileContext(nc) as tc:
        with (
            tc.tile_pool(name="persist", bufs=1) as pp,
            tc.tile_pool(name="gath", bufs=3) as gp,
            tc.tile_pool(name="work", bufs=3) as wk,
            tc.tile_pool(name="tiny", bufs=4) as tn,
            tc.tile_pool(name="psum", bufs=2, space="PSUM") as ps,
            tc.tile_pool(name="dram", bufs=1, space="DRAM") as dr,
        ):
            # ---- persistent SBUF state ----
            h = pp.tile([128, W, D], F32, tag="h")
            idx = pp.tile([128, IDXCOLS], I32, tag="idx")
            wn = pp.tile([128, KCOLS], F32, tag="wn")      # weights -> norm
            wst = pp.tile([128, L * D], F32, tag="wst")
            biasb = pp.tile([128, L * D], F32, tag="biasb")
            gamb = pp.tile([128, L * D], F32, tag="gamb")
            betb = pp.tile([128, L * D], F32, tag="betb")
            ident = pp.tile([128, D], F32, tag="ident")
            dinv = pp.tile([128, W], F32, tag="dinv")

            nc.sync.dma_start(out=h[:, :, :],
                              in_=x_in[:].rearrange("(w p) f -> p w f", p=128))
            nc.sync.dma_start(out=idx[:, :], in_=idx_in[:, :])
            nc.sync.dma_start(out=wn[:, :], in_=w_in[:, :])
            for l in range(L):
                for dst_t, src_t in ((wst, wst_in), (biasb, bias_in),
                                     (gamb, gam_in), (betb, bet_in)):
                    nc.sync.dma_start(out=dst_t[:, l * D:(l + 1) * D],
                                      in_=src_t[l, :, :])
            nc.sync.dma_start(out=ident[:, :], in_=id_in[:, :])

            # ---- degree -> dinv (once; includes self-loop weights) ----
            deg = tn.tile([128, W], F32, tag="deg")
            off = 0
            for w in range(W):
                blk = wn[:, off:off + KT[w]]
                nc.vector.tensor_reduce(deg[:, w:w + 1], blk, AX.X, ALU.add)
                off += KT[w]
            rdeg = tn.tile([128, W], F32, tag="rdeg")
            nc.vector.reciprocal(rdeg[:, :], deg[:, :])
            nc.scalar.sqrt(dinv[:, :], rdeg[:, :])
            # norm = w * dinv[dest]  (in place on wn)
            off = 0
            for w in range(W):
                nc.vector.tensor_scalar_mul(
                    wn[:, off:off + KT[w]], wn[:, off:off + KT[w]],
                    dinv[:, w:w + 1])
                off += KT[w]

            # ---- per-layer DRAM tables (double buffered across layers) ----
            tables = [dr.tile([NT, D], F32, name=f"table{i}", tag=f"table{i}") for i in range(2)]
            xw_own = [dr.tile([NP, D], F32, name=f"xwown{i}", tag=f"xwown{i}") for i in range(2)]

            for li in range(L):
                tab = tables[li % 2]
                own = xw_own[li % 2]
                wst_l = wst[:, li * D:(li + 1) * D]
                # -- build own table shard: T = dinv * (h @ Ws^T) --
                for w in range(W):
                    hT = ps.tile([128, D], F32, tag="hT")
                    nc.tensor.transpose(hT[:, :], h[:, w, :], ident[:, :])
                    hTs = wk.tile([128, D], F32, tag="hTs")
                    nc.scalar.activation(hTs[:, :], hT[:, :], ACTF.Copy)
                    mm = ps.tile([128, D], F32, tag="mm")
                    nc.tensor.matmul(mm[:, :], hTs[:, :], wst_l)
                    xw = wk.tile([128, D], F32, tag="xw")
                    nc.scalar.activation(xw[:, :], mm[:, :], ACTF.Copy,
                                         scale=dinv[:, w:w + 1])
                    nc.sync.dma_start(out=own[w * 128:(w + 1) * 128, :],
                                      in_=xw[:, :])
                nc.gpsimd.collective_compute(
                    "AllGather", ALU.bypass,
                    replica_groups=[list(range(C))],
                    ins=[own[:].opt()], outs=[tab[:].opt()])

                # -- aggregate into owned dests --
                off_k = 0
                off_i = 0
                for w in range(W):
                    ka, kb = int(KA[w]), int(KB[w])
                    kt = ka + kb
                    g = gp.tile([128, kt, D], F32, tag="g")
                    for k in range(kt):
                        nc.gpsimd.indirect_dma_start(
                            out=g[:, k, :], out_offset=None,
                            in_=tab[:, :],
                            in_offset=bass.IndirectOffsetOnAxis(
                                ap=idx[:, off_i + k:off_i + k + 1], axis=0))
                    # scale by per-(dest,k) norm, broadcast over features
                    nw = wn[:, off_k:off_k + kt].unsqueeze(2)
                    nc.vector.tensor_tensor(
                        g[:, :, :], g[:, :, :],
                        nw.broadcast_to([128, kt, D]), ALU.mult)
                    # reduce over k (strided innermost)
                    agg = wk.tile([128, D], F32, tag="agg")
                    nc.vector.tensor_reduce(
                        agg[:, :], g[:, :, :].transpose([0, 2, 1]),
                        AX.X, ALU.add)
                    # x0 = agg*dinv + bias
                    x0 = wk.tile([128, D], F32, tag="x0")
                    nc.vector.tensor_scalar_mul(x0[:, :], agg[:, :],
                                                dinv[:, w:w + 1])
                    nc.vector.tensor_add(x0[:, :], x0[:, :],
                                         biasb[:, li * D:(li + 1) * D])
                    # layernorm
                    sx = tn.tile([128, 1], F32, tag="sx")
                    nc.vector.tensor_reduce(sx[:, :], x0[:, :], AX.X, ALU.add)
                    sq = tn.tile([128, 1], F32, tag="sq")
                    sqs = wk.tile([128, D], F32, tag="sqs")
                    nc.scalar.activation(sqs[:, :], x0[:, :], ACTF.Square,
                                         accum_out=sq[:, :])
                    mu = tn.tile([128, 1], F32, tag="mu")
                    nc.vector.tensor_scalar_mul(mu[:, :], sx[:, :], 1.0 / D)
                    ms = tn.tile([128, 1], F32, tag="ms")
                    nc.vector.tensor_scalar(ms[:, :], sq[:, :], 1.0 / D,
                                            1e-5, ALU.mult, ALU.add)
                    mu2 = tn.tile([128, 1], F32, tag="mu2")
                    nc.vector.tensor_mul(mu2[:, :], mu[:, :], mu[:, :])
                    var = tn.tile([128, 1], F32, tag="var")
                    nc.vector.tensor_sub(var[:, :], ms[:, :], mu2[:, :])
                    rv = tn.tile([128, 1], F32, tag="rv")
                    nc.vector.reciprocal(rv[:, :], var[:, :])
                    rstd = tn.tile([128, 1], F32, tag="rstd")
                    nc.scalar.sqrt(rstd[:, :], rv[:, :])
                    nmr = tn.tile([128, 1], F32, tag="nmr")
                    nc.vector.tensor_mul(nmr[:, :], mu[:, :], rstd[:, :])
                    t = wk.tile([128, D], F32, tag="t")
                    nc.vector.tensor_scalar(t[:, :], x0[:, :], rstd[:, :],
                                            nmr[:, :], ALU.mult, ALU.subtract)
                    nc.vector.tensor_mul(t[:, :], t[:, :],
                                         gamb[:, li * D:(li + 1) * D])
                    nc.vector.tensor_add(t[:, :], t[:, :],
                                         betb[:, li * D:(li + 1) * D])
                    if li < L - 1:
                        nc.scalar.activation(t[:, :], t[:, :], ACTF.Relu)
                    nc.vector.tensor_add(h[:, w, :], t[:, :], h[:, w, :])
                    off_k += kt
                    off_i += kt

            nc.sync.dma_start(out=out_t[:].rearrange("(w p) f -> p w f", p=128),
                              in_=h[:, :, :])

    nc.compile()
    return nc


# ----------------------------------------------------------------------------
# Full kernel entry
# ----------------------------------------------------------------------------

def _kernel_impl(x, edge_index, edge_weight, Ws, bs, gammas, betas,
                 C=8, W=49, HALF=1 << 60, trace=False):
    N, D = x.shape
    L = Ws.shape[0]
    st = build_structure(edge_index, N, C, W, HALF)
    NP, NPC = st["NP"], st["NPC"]

    w2 = np.concatenate([np.asarray(edge_weight, dtype=np.float32),
                         np.ones(N, dtype=np.float32)])

    ident = np.eye(D, dtype=np.float32)
    wst = np.ascontiguousarray(np.transpose(np.asarray(Ws), (0, 2, 1)))
    bias_b = np.ascontiguousarray(
        np.broadcast_to(np.asarray(bs)[:, None, :], (L, D, D))).astype(np.float32)
    gam_b = np.ascontiguousarray(
        np.broadcast_to(np.asarray(gammas)[:, None, :], (L, D, D))).astype(np.float32)
    bet_b = np.ascontiguousarray(
        np.broadcast_to(np.asarray(betas)[:, None, :], (L, D, D))).astype(np.float32)

    in_maps = []
    for c in range(C):
        idx_img, w_img = pack_core(st, c, w2)
        xs = np.zeros((NP, D), dtype=np.float32)
        lo = c * NPC
        xs[st["rank"][lo:lo + NPC]] = np.asarray(x[lo:lo + NPC], dtype=np.float32)
        in_maps.append(dict(x_shard=xs, idx_img=idx_img, w_img=w_img,
                            wst=wst, bias_b=bias_b, gamma_b=gam_b,
                            beta_b=bet_b, ident=ident))

    nc = build_program(st, L, D)
    res = run_bass_kernel_spmd(nc, in_maps, list(range(C)), trace=trace)

    out = np.empty((N, D), dtype=np.float32)
    for c in range(C):
        lo = c * NPC
        sh = res.results[c]["out_shard"]
        out[lo:lo + NPC] = sh[st["rank"][lo:lo + NPC]]
    return out, res


def kernel(x, edge_index, edge_weight, Ws, bs, gammas, betas):
    out, _ = _kernel_impl(np.asarray(x), np.asarray(edge_index),
                          np.asarray(edge_weight), np.asarray(Ws),
                          np.asarray(bs), np.asarray(gammas),
                          np.asarray(betas))
    return out



# revision 21
# speedup vs baseline: 2.8115x; 2.8115x over previous
"""GCN encoder (3-layer GCNConv + LayerNorm + ReLU + residual) on 8 TRN2
NeuronCores via Bass/Tile.

Sharding: nodes are partitioned across cores (graph parallel). Each core owns
NPC nodes; per-layer the full (dinv-scaled) xw table is AllGathered to every
core's DRAM, then each core pulls its in-edge source rows with batched
dma_gather (SWDGE), scales by edge weight, and segment-reduces into its owned
destinations.

Gather indices are int16, so the 50176-row table is addressed in three
phases with base row offsets 0, (NT-32768)/2 and NT-32768, each covering
32768 rows. Edges in overlap zones are assigned to phases per-window via a
small LP + per-dest greedy water-fill that minimizes total padded slots.
"""

import numpy as np

import concourse.bacc as bacc
import concourse.bass as bass
import concourse.mybir as mybir
from concourse.tile import TileContext
from concourse.bass_utils import run_bass_kernel_spmd
from concourse.library_config import mlp

F32 = mybir.dt.float32
BF16 = mybir.dt.bfloat16
I16 = mybir.dt.int16
AX = mybir.AxisListType
ALU = mybir.AluOpType
ACTF = mybir.ActivationFunctionType

HALF = 32768  # int16-addressable rows per gather base
KMAX = 8      # slots per dma_gather call (1024 descriptors, fits the ring)
NQ = 4        # SWDGE queues (calls round-robin to reduce ring stalls)


# ----------------------------------------------------------------------------
# Host-side structure packing (pure index/layout manipulation + reordering)
# ----------------------------------------------------------------------------

def build_structure(edge_index, N, C, W):
    """Partition nodes across C cores, degree-sort each core's dests into
    windows of 128, split each dest's in-edges into three gather phases
    (base rows 0 / B2 / B3, each covering 32768 rows) and compute per-window
    padded slot counts KP[phase] (maxed over cores so the SPMD program is
    identical on all cores)."""
    NPC = N // C              # owned (real) nodes per core
    NP = W * 128              # padded nodes per core
    NT = NP * C               # table rows
    B3 = NT - HALF            # phase-2 base row
    B2 = B3 // 2              # phase-1 base row
    bases = (0, B2, B3)
    src = edge_index[0].astype(np.int64)
    dst = edge_index[1].astype(np.int64)
    E = src.shape[0]

    # append self loops (weight handled separately by caller: w=1)
    loop = np.arange(N, dtype=np.int64)
    src2 = np.concatenate([src, loop])
    dst2 = np.concatenate([dst, loop])
    eid2 = np.arange(E + N, dtype=np.int64)   # index into w2 = [edge_weight, ones]

    owner = dst2 // NPC                        # dest core of each edge
    deg_all = np.bincount(dst2, minlength=N)   # per-dest slot count (incl self)

    # per-core permutation: sort owned dests by degree desc (stable)
    rank = np.empty(N, dtype=np.int64)         # local rank of node on its owner
    for c in range(C):
        lo, hi = c * NPC, (c + 1) * NPC
        order = np.argsort(-deg_all[lo:hi], kind="stable")
        rank[lo + order] = np.arange(NPC)
    node_pos = (np.arange(N) // NPC) * NP + rank      # table row of each node

    def wmax(v):
        return v.reshape(W, 128).max(axis=1)

    # zones: z1 [0,B2) only P0; z2 [B2,B3) P0/P1; z3 [B3,HALF) any;
    # z4 [HALF,B2+HALF) P1/P2; z5 [B2+HALF,NT) only P2.
    cores = []
    M1 = np.zeros(W, dtype=np.int64); M5 = np.zeros(W, dtype=np.int64)
    M12 = np.zeros(W, dtype=np.int64); M45 = np.zeros(W, dtype=np.int64)
    Md = np.zeros(W, dtype=np.int64); M15 = np.zeros(W, dtype=np.int64)
    for c in range(C):
        sel = owner == c
        spos = node_pos[src2[sel]]            # table row of source
        e_id = eid2[sel]
        dloc = rank[dst2[sel]]                # local dest rank [0, NPC)
        zcls = np.digitize(spos, [B2, B3, HALF, B2 + HALF])  # 0..4 = z1..z5
        bc = lambda m: np.bincount(dloc[m], minlength=NP)
        z = [bc(zcls == i) for i in range(5)]
        deg = z[0] + z[1] + z[2] + z[3] + z[4]
        M1 = np.maximum(M1, wmax(z[0]))
        M5 = np.maximum(M5, wmax(z[4]))
        M12 = np.maximum(M12, wmax(z[0] + z[1]))
        M45 = np.maximum(M45, wmax(z[3] + z[4]))
        Md = np.maximum(Md, wmax(deg))
        M15 = np.maximum(M15, wmax(z[0] + z[4]))
        cores.append(dict(spos=spos, eid=e_id, dloc=dloc, zcls=zcls,
                          z=z, deg=deg))

    # per-window caps (a, m, b) minimizing total slots subject to coverage
    KP = np.zeros((3, W), dtype=np.int64)
    for w in range(W):
        best = None
        for a in range(int(M1[w]), int(M12[w]) + 1):
            for b in range(int(M5[w]), int(M45[w]) + 1):
                if a + b < M15[w]:
                    continue
                m = max(int(M12[w]) - a, int(M45[w]) - b,
                        int(Md[w]) - a - b, 0)
                s = a + m + b
                if best is None or s < best[0]:
                    best = (s, a, m, b)
        if best is None:
            best = (0, 0, 0, 0)
        KP[0, w], KP[1, w], KP[2, w] = best[1], best[2], best[3]

    # per-core, per-dest phase assignment (greedy water-fill within caps)
    acap = np.repeat(KP[0], 128)
    bcap = np.repeat(KP[2], 128)
    mcap = np.repeat(KP[1], 128)
    for c in range(C):
        d = cores[c]
        z1, z2, z3, z4, z5 = d["z"]
        t02 = np.minimum(z2, acap - z1)               # z2 -> P0
        t42 = np.minimum(z4, bcap - z5)               # z4 -> P2
        t30 = np.minimum(z3, acap - z1 - t02)         # z3 -> P0
        t32 = np.minimum(z3 - t30, bcap - z5 - t42)   # z3 -> P2
        cnt0 = z1 + t02 + t30
        cnt2 = z5 + t42 + t32
        cnt1 = d["deg"] - cnt0 - cnt2
        assert (cnt1 <= mcap).all() and (cnt0 <= acap).all() \
            and (cnt2 <= bcap).all()

        # per-edge phase: ordinal within (dest, zone) vs thresholds
        dloc, zcls = d["dloc"], d["zcls"]
        key = dloc * 5 + zcls
        o = np.argsort(key, kind="stable")
        ks = key[o]
        kcnt = np.bincount(ks, minlength=NP * 5)
        kstart = np.zeros(NP * 5, dtype=np.int64)
        kstart[1:] = np.cumsum(kcnt)[:-1]
        ordinal = np.arange(len(ks)) - kstart[ks]
        phase_s = np.empty(len(ks), dtype=np.int64)
        zs = zcls[o]; dl = dloc[o]
        phase_s[zs == 0] = 0
        phase_s[zs == 4] = 2
        m_ = zs == 1
        phase_s[m_] = np.where(ordinal[m_] < t02[dl[m_]], 0, 1)
        m_ = zs == 3
        phase_s[m_] = np.where(ordinal[m_] < t42[dl[m_]], 2, 1)
        m_ = zs == 2
        om = ordinal[m_]; dm = dl[m_]
        phase_s[m_] = np.where(om < t30[dm], 0,
                               np.where(om < t30[dm] + t32[dm], 2, 1))

        # final edge order: (dest, phase)
        spos_s = d["spos"][o]; eid_s = d["eid"][o]
        o2 = np.lexsort((phase_s, dl))
        starts = np.zeros(NP, dtype=np.int64)
        starts[1:] = np.cumsum(d["deg"])[:-1]
        cores[c] = dict(spos=spos_s[o2], eid=eid_s[o2],
                        cnt=(cnt0, cnt1, cnt2), starts=starts)

    return dict(NPC=NPC, NP=NP, NT=NT, bases=bases, C=C, W=W, KP=KP,
                cores=cores, rank=rank, node_pos=node_pos)


def _pad_block(vals, starts, lens, K, fill):
    """[128] ragged segments of `vals` -> padded [128, K] with `fill`."""
    col = np.arange(K)[None, :]
    mask = col < lens[:, None]
    sp = starts[:, None] + col
    sp = np.where(mask, sp, 0)
    out = np.where(mask, vals[sp], fill)
    return out


def _wrap16(flat):
    """flat [n] -> [128, n//16] int16: index i at partition i%16, col i//16,
    replicated down the 8 sixteen-partition stripes."""
    n = flat.shape[0]
    assert n % 16 == 0
    img = flat.reshape(n // 16, 16).T          # [16, n//16]
    return np.tile(img, (8, 1)).astype(np.int16)


def pack_core(st, c, w2):
    """Build the wrapped int16 index image and f32 weight image for core c.

    Layout per window w: phase 0 block (KP[0,w] slots), then phase 1, then
    phase 2. Weight image: [128, sum(KP)] with col = slot, row = dest.
    Index image: per phase-block the flat k-major list (i = k*128+d)
    wrapped 16-wide -> [128, 8*K] int16, blocks concatenated."""
    W, KP, bases = st["W"], st["KP"], st["bases"]
    d = st["cores"][c]
    spos, eid = d["spos"], d["eid"]
    cnt0, cnt1, cnt2 = d["cnt"]
    starts = d["starts"]
    wvals = w2[eid]

    idx_cols = []
    w_cols = []
    for w in range(W):
        vs = slice(w * 128, (w + 1) * 128)
        stt = starts[vs]
        c0, c1, c2 = cnt0[vs], cnt1[vs], cnt2[vs]
        for K, s0, ln, off in ((int(KP[0, w]), stt, c0, bases[0]),
                               (int(KP[1, w]), stt + c0, c1, bases[1]),
                               (int(KP[2, w]), stt + c0 + c1, c2, bases[2])):
            if K == 0:
                continue
            pi = _pad_block(spos, s0, ln, K, off).astype(np.int64) - off
            pw = _pad_block(wvals, s0, ln, K, 0.0)
            assert pi.min() >= 0 and pi.max() < HALF
            idx_cols.append(_wrap16(pi.T.reshape(-1)))    # [128, 8K]
            w_cols.append(pw.astype(np.float32))          # [128, K]
    idx_img = np.concatenate(idx_cols, axis=1)
    w_img = np.concatenate(w_cols, axis=1)
    return idx_img, w_img


# ----------------------------------------------------------------------------
# Bass program
# ----------------------------------------------------------------------------

def build_program(st, L, D=128):
    W = st["W"]
    NP = st["NP"]
    NT = st["NT"]
    bases = st["bases"]
    C = st["C"]
    KP = st["KP"]
    KT = [int(KP[:, w].sum()) for w in range(W)]
    KCOLS = int(sum(KT))
    IDXW = 8 * KCOLS

    nc = bacc.Bacc("TRN2", target_bir_lowering=False, debug=True,
                   num_swdge_queues=NQ)

    x_in = nc.dram_tensor("x_shard", [NP, D], F32, kind="ExternalInput")
    idx_in = nc.dram_tensor("idx_img", [128, IDXW], I16, kind="ExternalInput")
    w_in = nc.dram_tensor("w_img", [128, KCOLS], F32, kind="ExternalInput")
    wst_in = nc.dram_tensor("wst", [L, D, D], F32, kind="ExternalInput")
    bias_in = nc.dram_tensor("bias_b", [L, D, D], F32, kind="ExternalInput")
    gam_in = nc.dram_tensor("gamma_b", [L, D, D], F32, kind="ExternalInput")
    bet_in = nc.dram_tensor("beta_b", [L, D, D], F32, kind="ExternalInput")
    id_in = nc.dram_tensor("ident", [D, D], F32, kind="ExternalInput")
    out_t = nc.dram_tensor("out_shard", [NP, D], F32, kind="ExternalOutput")

    with TileContext(nc) as tc:
        with (
            tc.tile_pool(name="persist", bufs=1) as pp,
            tc.tile_pool(name="gath", bufs=3) as gp,
            tc.tile_pool(name="work", bufs=3) as wk,
            tc.tile_pool(name="tiny", bufs=4) as tn,
            tc.tile_pool(name="psum", bufs=2, space="PSUM") as ps,
            tc.tile_pool(name="dram", bufs=1, space="DRAM") as dr,
        ):
            nc.gpsimd.load_library(mlp)

            # ---- persistent SBUF state ----
            h = pp.tile([128, W, D], F32, tag="h")
            idx = pp.tile([128, IDXW], I16, tag="idx")
            wn = pp.tile([128, KCOLS], F32, tag="wn")      # weights -> norm
            wst = pp.tile([128, L * D], F32, tag="wst")
            biasb = pp.tile([128, L * D], F32, tag="biasb")
            gamb = pp.tile([128, L * D], F32, tag="gamb")
            betb = pp.tile([128, L * D], F32, tag="betb")
            ident = pp.tile([128, D], F32, tag="ident")
            dinv = pp.tile([128, W], F32, tag="dinv")

            nc.sync.dma_start(out=h[:, :, :],
                              in_=x_in[:].rearrange("(w p) f -> p w f", p=128))
            nc.sync.dma_start(out=idx[:, :], in_=idx_in[:, :])
            nc.sync.dma_start(out=wn[:, :], in_=w_in[:, :])
            for l in range(L):
                for dst_t, src_t in ((wst, wst_in), (biasb, bias_in),
                                     (gamb, gam_in), (betb, bet_in)):
                    nc.sync.dma_start(out=dst_t[:, l * D:(l + 1) * D],
                                      in_=src_t[l, :, :])
            nc.sync.dma_start(out=ident[:, :], in_=id_in[:, :])

            # ---- degree -> dinv (once; includes self-loop weights) ----
            deg = tn.tile([128, W], F32, tag="deg")
            off = 0
            for w in range(W):
                blk = wn[:, off:off + KT[w]]
                nc.vector.tensor_reduce(deg[:, w:w + 1], blk, AX.X, ALU.add)
                off += KT[w]
            rdeg = tn.tile([128, W], F32, tag="rdeg")
            nc.vector.reciprocal(rdeg[:, :], deg[:, :])
            nc.scalar.sqrt(dinv[:, :], rdeg[:, :])
            # norm = w * dinv[dest]  (in place on wn), then cast to bf16
            wnb = pp.tile([128, KCOLS], BF16, tag="wnb")
            off = 0
            for w in range(W):
                nc.vector.tensor_scalar_mul(
                    wn[:, off:off + KT[w]], wn[:, off:off + KT[w]],
                    dinv[:, w:w + 1])
                off += KT[w]
            nc.vector.tensor_copy(wnb[:, :], wn[:, :])

            # ---- per-layer DRAM tables (bf16) ----
            tables = [dr.tile([NT, D], BF16, name=f"table{i}", tag=f"table{i}")
                      for i in range(L)]
            xw_own = [dr.tile([NP, D], BF16, name=f"xwown{i}", tag=f"xwown{i}")
                      for i in range(2)]

            self_qn = [0]
            for li in range(L):
                tab = tables[li]
                own = xw_own[li % 2]
                wst_l = wst[:, li * D:(li + 1) * D]
                # -- build own table shard: T = dinv * (h @ Ws^T) --
                for w in range(W):
                    hT = ps.tile([128, D], F32, tag="hT")
                    nc.tensor.transpose(hT[:, :], h[:, w, :], ident[:, :])
                    hTs = wk.tile([128, D], F32, tag="hTs")
                    nc.scalar.activation(hTs[:, :], hT[:, :], ACTF.Copy)
                    mm = ps.tile([128, D], F32, tag="mm")
                    nc.tensor.matmul(mm[:, :], hTs[:, :], wst_l)
                    xw = wk.tile([128, D], BF16, tag="xw")
                    nc.scalar.activation(xw[:, :], mm[:, :], ACTF.Copy,
                                         scale=dinv[:, w:w + 1])
                    nc.sync.dma_start(out=own[w * 128:(w + 1) * 128, :],
                                      in_=xw[:, :])
                nc.gpsimd.collective_compute(
                    "AllGather", ALU.bypass,
                    replica_groups=[list(range(C))],
                    ins=[own[:].opt()], outs=[tab[:].opt()])

                # -- aggregate into owned dests --
                off_k = 0
                off_i = 0
                for w in range(W):
                    kp = [int(KP[p, w]) for p in range(3)]
                    kt = sum(kp)
                    g = gp.tile([128, kt, D], BF16, tag="g")
                    klo = 0
                    ilo = off_i
                    for p in range(3):
                        for c0 in range(0, kp[p], KMAX):
                            k = min(KMAX, kp[p] - c0)
                            nc.gpsimd.dma_gather(
                                g[:, klo:klo + k, :],
                                tab[bases[p]:bases[p] + HALF, :],
                                idx[:, ilo:ilo + 8 * k],
                                128 * k, 128 * k, D, queue_num=self_qn[0])
                            self_qn[0] = (self_qn[0] + 1) % NQ
                            klo += k
                            ilo += 8 * k
                    # scale by per-(dest,k) norm, broadcast over features
                    nw = wnb[:, off_k:off_k + kt].unsqueeze(2)
                    nc.vector.tensor_tensor(
                        g[:, :, :], g[:, :, :],
                        nw.broadcast_to([128, kt, D]), ALU.mult)
                    # reduce over k: contiguous halving tree (bf16), final
                    # level into f32
                    kk = kt
                    while kk > 2:
                        h2 = kk // 2
                        nc.vector.tensor_add(g[:, :h2, :], g[:, :h2, :],
                                             g[:, kk - h2:kk, :])
                        kk -= h2
                    agg = wk.tile([128, D], F32, tag="agg")
                    if kk == 2:
                        nc.vector.tensor_add(agg[:, :], g[:, 0, :],
                                             g[:, 1, :])
                    else:
                        nc.vector.tensor_copy(agg[:, :], g[:, 0, :])
                    # x0 = agg*dinv + bias (fused)
                    x0 = wk.tile([128, D], F32, tag="x0")
                    nc.vector.scalar_tensor_tensor(
                        x0[:, :], agg[:, :], dinv[:, w:w + 1],
                        biasb[:, li * D:(li + 1) * D], ALU.mult, ALU.add)
                    # layernorm
                    sx = tn.tile([128, 1], F32, tag="sx")
                    nc.vector.tensor_reduce(sx[:, :], x0[:, :], AX.X, ALU.add)
                    sq = tn.tile([128, 1], F32, tag="sq")
                    sqs = wk.tile([128, D], F32, tag="sqs")
                    nc.scalar.activation(sqs[:, :], x0[:, :], ACTF.Square,
                                         accum_out=sq[:, :])
                    mu = tn.tile([128, 1], F32, tag="mu")
                    nc.vector.tensor_scalar_mul(mu[:, :], sx[:, :], 1.0 / D)
                    ms = tn.tile([128, 1], F32, tag="ms")
                    nc.vector.tensor_scalar(ms[:, :], sq[:, :], 1.0 / D,
                                            1e-5, ALU.mult, ALU.add)
                    mu2 = tn.tile([128, 1], F32, tag="mu2")
                    nc.vector.tensor_mul(mu2[:, :], mu[:, :], mu[:, :])
                    var = tn.tile([128, 1], F32, tag="var")
                    nc.vector.tensor_sub(var[:, :], ms[:, :], mu2[:, :])
                    rv = tn.tile([128, 1], F32, tag="rv")
                    nc.vector.reciprocal(rv[:, :], var[:, :])
                    rstd = tn.tile([128, 1], F32, tag="rstd")
                    nc.scalar.sqrt(rstd[:, :], rv[:, :])
                    nmr = tn.tile([128, 1], F32, tag="nmr")
                    nc.vector.tensor_mul(nmr[:, :], mu[:, :], rstd[:, :])
                    t = wk.tile([128, D], F32, tag="t")
                    nc.vector.tensor_scalar(t[:, :], x0[:, :], rstd[:, :],
                                            nmr[:, :], ALU.mult, ALU.subtract)
                    nc.vector.tensor_mul(t[:, :], t[:, :],
                                         gamb[:, li * D:(li + 1) * D])
                    nc.vector.tensor_add(t[:, :], t[:, :],
                                         betb[:, li * D:(li + 1) * D])
                    if li < L - 1:
                        nc.scalar.activation(t[:, :], t[:, :], ACTF.Relu)
                    nc.vector.tensor_add(h[:, w, :], t[:, :], h[:, w, :])
                    off_k += kt
                    off_i += 8 * kt

            nc.sync.dma_start(out=out_t[:].rearrange("(w p) f -> p w f", p=128),
                              in_=h[:, :, :])

    nc.compile()
    return nc


# ----------------------------------------------------------------------------
# Full kernel entry
# ----------------------------------------------------------------------------

def _kernel_impl(x, edge_index, edge_weight, Ws, bs, gammas, betas,
                 C=8, W=49, trace=False):
    N, D = x.shape
    L = Ws.shape[0]
    st = build_structure(edge_index, N, C, W)
    NP, NPC = st["NP"], st["NPC"]

    w2 = np.concatenate([np.asarray(edge_weight, dtype=np.float32),
                         np.ones(N, dtype=np.float32)])

    ident = np.eye(D, dtype=np.float32)
    wst = np.ascontiguousarray(np.transpose(np.asarray(Ws), (0, 2, 1)))
    bias_b = np.ascontiguousarray(
        np.broadcast_to(np.asarray(bs)[:, None, :], (L, D, D))).astype(np.float32)
    gam_b = np.ascontiguousarray(
        np.broadcast_to(np.asarray(gammas)[:, None, :], (L, D, D))).astype(np.float32)
    bet_b = np.ascontiguousarray(
        np.broadcast_to(np.asarray(betas)[:, None, :], (L, D, D))).astype(np.float32)

    in_maps = []
    for c in range(C):
        idx_img, w_img = pack_core(st, c, w2)
        xs = np.zeros((NP, D), dtype=np.float32)
        lo = c * NPC
        xs[st["rank"][lo:lo + NPC]] = np.asarray(x[lo:lo + NPC], dtype=np.float32)
        in_maps.append(dict(x_shard=xs, idx_img=idx_img, w_img=w_img,
                            wst=wst, bias_b=bias_b, gamma_b=gam_b,
                            beta_b=bet_b, ident=ident))

    nc = build_program(st, L, D)
    res = run_bass_kernel_spmd(nc, in_maps, list(range(C)), trace=trace)

    out = np.empty((N, D), dtype=np.float32)
    for c in range(C):
        lo = c * NPC
        sh = res.results[c]["out_shard"]
        out[lo:lo + NPC] = sh[st["rank"][lo:lo + NPC]]
    return out, res


def kernel(x, edge_index, edge_weight, Ws, bs, gammas, betas):
    out, _ = _kernel_impl(np.asarray(x), np.asarray(edge_index),
                          np.asarray(edge_weight), np.asarray(Ws),
                          np.asarray(bs), np.asarray(gammas),
                          np.asarray(betas))
    return out


if __name__ == "__main__":
    # quick packing sanity check
    import json
    rng = np.random.default_rng(0)
    N, E, C, W = 50000, 1600000, 8, 49
    ei = np.stack([rng.integers(0, N, E), rng.integers(0, N, E)]).astype(np.int32)
    st = build_structure(ei, N, C, W)
    kt = st["KP"].sum(axis=0)
    print("KT per window:", kt.tolist())
    print("KCOLS:", int(kt.sum()))
